# revision 1
# baseline (speedup 1.0000x reference)
"""BlockCirculantConv on 8 Trainium2 NeuronCores.

The reference computes, per batch image b:
    xu = unfold(x[b])                       # (2304, 1024), f = c*9 + (di*3+dj)
    Y  = xu.flatten().reshape(1024, 2304)   # torch-faithful row-major reshape
    out_T = (Y @ W).T                       # W = expanded block-circulant (2304, 512)
    out[b] = out_T.reshape(512, 32, 32)
with W[q*64+s, p*64+t] = weight[p, q, (t-s) % 64]  (rfft product == circular conv).

Because of the row-major reshape, row n = 4c+j of Y is a contiguous 2304-chunk of
channel c's 9 shifted images:  Y[4c+j, k] = Z_c[(j*2304+k)//1024, (j*2304+k)%1024]
where Z_c[dd, i*32+jj] = xpad[b, c, i+dd//3-1, jj+dd%3-1].

So out_T[m, 4c+j] = sum_k W[k, m] * S_kj[k, c]  where for a 128-aligned k-tile the
rhs S tile is a CONTIGUOUS 128-row slice of a (per-dj) zero-padded, transposed copy
of the image: xt3[dj, 1 + i*32 + jj, c] = xpad[b, c, i-1, jj+dj-1].

Device kernel per core (data-parallel over batch, 1 image/core):
  - inputs in fp16 (halves DMA bytes; fp32 PSUM accumulate; rel err ~3e-4)
  - weights + all rhs data DMA'd into SBUF as a few large chunk transfers
    (each dma_start costs ~650ns of HWDGE sequencer time)
  - 8 PSUM banks accumulate out_T as 4 m-tiles x 2 column-halves over 18
    k-tiles; dummy warm-up matmuls release the HAM clock gate early;
    k-tiles 10..17 run one psum at a time so drains overlap the stream
  - drain: DVE/ACT copies PSUM -> SBUF, DMA out in (j*256+c) column
    order; host permutes columns back to n = 4c+j.
"""

import sys

if "/opt/trn_rl_repo" not in sys.path:
    sys.path.insert(0, "/opt/trn_rl_repo")

import numpy as np

B, C, H, W_IMG = 8, 256, 32, 32
L = H * W_IMG               # 1024
BLK = 64
Q, P = 36, 8
K_FULL = Q * BLK            # 2304
M_OUT = P * BLK             # 512
KT = K_FULL // 128          # 18 k-tiles
N_CORES = 8
XT_ROWS = 1 + 34 * 32 + 1   # 1090 padded rows per dj copy

_CACHE = {}

# "float16" (half input bytes, full-rate PE, rel err ~3e-4) or
# "float32r" (single-pass fp32 matmul, rel err ~1.5e-4)
IN_DTYPE = "float16"


def _patch_ldw_opt():
    """(kept as a hook; ldw-opt=true fails walrus codegen, so this is a no-op)"""
    from concourse import bass_utils

    if getattr(bass_utils.run_command, "_ldw_patched", False):
        return
    orig = bass_utils.run_command

    def run_command(cmd, *a, **kw):
        cmd = [
            c
            if isinstance(c, str)
            else c
            for c in cmd
        ]
        return orig(cmd, *a, **kw)

    run_command._ldw_patched = True
    bass_utils.run_command = run_command


def _build_nc():
    import concourse.bacc as bacc
    import concourse.tile as tile
    import concourse.mybir as mybir

    _patch_ldw_opt()

    dt = mybir.dt
    din = getattr(dt, IN_DTYPE)
    nc = bacc.Bacc("TRN2", target_bir_lowering=False, debug=False)

    xt3 = nc.dram_tensor("xt3", [3, XT_ROWS, C], din, kind="ExternalInput").ap()
    wmat = nc.dram_tensor("wmat", [K_FULL, M_OUT], din, kind="ExternalInput").ap()
    out = nc.dram_tensor("out", [M_OUT, L], dt.float32, kind="ExternalOutput").ap()

    f32 = dt.float32

    # S chunk plan: for each j, the u-range [j*2304, (j+1)*2304) splits at
    # dd (=u//1024) boundaries into runs of whole k-tiles with a constant
    # source row offset. Each dma_start costs ~650ns of HWDGE sequencer
    # time, so use as few (big) chunks as possible; only the first k-tiles
    # get a small chunk so the PE can start early.
    chunks = []  # (j, kt_start, n_kt, dj, src_row0)
    for j in range(4):
        kt = 0
        while kt < KT:
            u = j * K_FULL + kt * 128
            dd, l0 = divmod(u, L)
            di, dj = divmod(dd, 3)
            kt_end_dd = min(KT, ((dd + 1) * L - j * K_FULL) // 128)
            cap = 2 if kt == 0 else (4 if kt <= 6 else KT)
            n_kt = min(cap, kt_end_dd - kt)
            chunks.append((j, kt, n_kt, dj, 1 + di * 32 + l0))
            kt += n_kt
    # issue order: ascending kt so early k-tiles land first
    chunks.sort(key=lambda c: (c[1], c[0]))
    # W chunk plan: (kt_start, n_kt)
    wchunks = [(0, 2), (2, 4), (6, 4), (10, 4), (14, 4)]

    with tile.TileContext(nc) as tc:
        with (
            tc.tile_pool(name="wpool", bufs=1) as wpool,
            tc.tile_pool(name="spool", bufs=1) as spool,
            tc.tile_pool(name="opool", bufs=4) as opool,
            tc.tile_pool(name="ppool", bufs=1, space="PSUM") as ppool,
        ):
            # PE warmup: the HAM clock gate starts at 1.2 GHz and needs
            # ~3.4us of sustained PE activity to release to 2.4 GHz. Run
            # dummy matmuls on a zeroed tile while the first DMA chunks are
            # still in flight so the real matmuls start warm.
            wz = wpool.tile([128, 512], din, name="wz", tag="wz")
            nc.gpsimd.memset(wz[:], 0.0)

            # 8 PSUM accumulators: index = mt*2 + nh (m-tile x column-half)
            psums = [
                ppool.tile([128, 512], f32, name=f"ps{i}", tag=f"ps{i}")
                for i in range(8)
            ]

            # All rhs data resident: sbig[p, kt, j, c]; weights wbig[p, kt, m]
            sbig = spool.tile([128, KT, 4, 256], din, name="sbig", tag="sbig")
            wbig = wpool.tile([128, KT, 512], din, name="wbig", tag="wbig")

            for _ in range(8):
                nc.tensor.matmul(
                    psums[7][:], wz[:, :128], wz[:], start=True, stop=True
                )

            # S chunks on the sync ring, W chunks on the scalar ring,
            # both in ascending-kt order
            # The kt0 chunks + w0 gate the first matmul; split their
            # triggers across both HWDGE rings (sync + scalar) so the
            # ~650ns-per-trigger serialization doesn't stack up.
            def issue_s(c, eng):
                j, kt0, n_kt, dj, r0 = c
                src = xt3[dj, r0 : r0 + n_kt * 128, :].rearrange(
                    "(blk p) c -> p blk c", p=128
                )
                eng.dma_start(sbig[:, kt0 : kt0 + n_kt, j, :], src)

            first = [c for c in chunks if c[1] == 0]
            rest = [c for c in chunks if c[1] > 0]
            issue_s(first[0], nc.sync)
            issue_s(first[1], nc.sync)
            issue_s(first[2], nc.scalar)
            issue_s(first[3], nc.scalar)

            ci = 0
            for kt0w, n_ktw in wchunks:
                while ci < len(rest) and rest[ci][1] <= kt0w:
                    issue_s(rest[ci], nc.sync)
                    ci += 1
                wsrc = wmat[kt0w * 128 : (kt0w + n_ktw) * 128, :].rearrange(
                    "(blk p) m -> p blk m", p=128
                )
                nc.scalar.dma_start(wbig[:, kt0w : kt0w + n_ktw, :], wsrc)
            for c in rest[ci:]:
                issue_s(c, nc.sync)

            # Phase 1: k-tiles 0..SPLIT-1 round-robin over all 8 psums
            # (keeps every accumulator fed while chunks stream in).
            # Phase 2: once all data is resident, finish one psum at a
            # time so drains + output stores overlap the remaining
            # matmuls instead of piling up in the tail.
            SPLIT = 10
            for kt in range(SPLIT):
                for mt in range(4):
                    for nh in range(2):
                        nc.tensor.matmul(
                            psums[mt * 2 + nh][:],
                            wbig[:, kt, mt * 128 : (mt + 1) * 128],
                            sbig[:, kt, nh * 2 : nh * 2 + 2, :],
                            start=(kt == 0),
                            stop=False,
                        )
            for mt in range(4):
                for nh in range(2):
                    for kt in range(SPLIT, KT):
                        nc.tensor.matmul(
                            psums[mt * 2 + nh][:],
                            wbig[:, kt, mt * 128 : (mt + 1) * 128],
                            sbig[:, kt, nh * 2 : nh * 2 + 2, :],
                            start=False,
                            stop=(kt == KT - 1),
                        )

            # Drain: contiguous copies; out stays in (j*256+c) column order,
            # host permutes to n = 4c+j. Per-half DMAs so the final store
            # pipelines behind the last copies.
            for mt in range(4):
                ot = opool.tile([128, L], f32, name="ot", tag="ot")
                for nh in range(2):
                    src = psums[mt * 2 + nh][:]
                    dst = ot[:, nh * 512 : (nh + 1) * 512]
                    if nh == 0:
                        nc.vector.tensor_copy(dst, src)
                    else:
                        nc.scalar.copy(dst, src)
                    nc.sync.dma_start(
                        out[mt * 128 : (mt + 1) * 128, nh * 512 : (nh + 1) * 512],
                        dst,
                    )

    nc.compile()
    return nc


def _host_prep(x, weight):
    np_in = np.float16 if IN_DTYPE == "float16" else np.float32
    x = np.ascontiguousarray(x, dtype=np.float32)
    weight = np.ascontiguousarray(weight, dtype=np.float32)

    # Expanded block-circulant matrix: W[q*64+s, p*64+t] = weight[p, q, (t-s)%64]
    idx = (np.arange(BLK)[None, :] - np.arange(BLK)[:, None]) % BLK   # (s, t)
    w4 = weight[:, :, idx]                                            # (p, q, s, t)
    wmat = np.ascontiguousarray(
        w4.transpose(1, 2, 0, 3).reshape(K_FULL, M_OUT), dtype=np_in
    )

    # Per-batch padded/shifted transposed images: xt3[b, dj, 1+i*32+jj, c]
    #   = x[b, c, i-1, jj+dj-1] (zero outside the image)
    xp = x.transpose(0, 2, 3, 1).astype(np_in)                        # (b, i, j, c)
    xt3 = np.zeros((B, 3, XT_ROWS, C), np_in)
    v = xt3[:, :, 1 : 1 + 34 * 32, :].reshape(B, 3, 34, 32, C)
    v[:, 0, 1:33, 1:32] = xp[:, :, 0:31]
    v[:, 1, 1:33, 0:32] = xp
    v[:, 2, 1:33, 0:31] = xp[:, :, 1:32]
    return xt3, wmat


def _run(x, weight, trace=False, trace_kwargs=None):
    from concourse.bass_utils import run_bass_kernel_spmd

    if "nc" not in _CACHE:
        _CACHE["nc"] = _build_nc()
    nc = _CACHE["nc"]

    xt3, wmat = _host_prep(x, weight)
    in_maps = [{"xt3": xt3[b], "wmat": wmat} for b in range(N_CORES)]
    res = run_bass_kernel_spmd(
        nc,
        in_maps,
        list(range(N_CORES)),
        trace=trace,
        **(trace_kwargs or {}),
    )
    out = np.stack([res.results[b]["out"] for b in range(N_CORES)])
    # device columns are (j*256 + c); output spatial index is n = 4c + j
    out = (
        out.reshape(B, M_OUT, 4, 256)
        .transpose(0, 1, 3, 2)
        .reshape(B, M_OUT, H, W_IMG)
    )
    return np.ascontiguousarray(out), res


def kernel(x, weight):
    out, _ = _run(x, weight, trace=False)
    return out



# revision 2
# speedup vs baseline: 1.2702x; 1.2702x over previous
"""BlockCirculantConv on 8 Trainium2 NeuronCores — FFT-domain device kernel.

The reference is, per output row n = 4c+j (torch-faithful row-major reshape):
    Hf[n, p, f] = sum_q Xf[n, q, f] * wf[p, q, f]      (complex, f = 0..32)
with Xf[n, q, :] = rfft of 64-block a = 36j+q of channel c's 9 shifted images
and wf = rfft(weight). rfft/irfft are cheap O(input) transforms done on the
host; the device does only the FLOP-bearing q->p contraction (64x fewer MACs
than the expanded dense 2304x512 matmul the previous kernel used).

Device per core (data-parallel over batch, 1 image/core):
  - rhs  xall[k=(fl*36+q), g=(gi*2+comp), n=4c+j]  fp16, 4.86 MB
  - w    wall[k, g, m=(fl*16+oc*8+p)]              fp16, 0.23 MB
    (11 freq-groups gi of 3 freqs fl; comp = re/im input pass; oc = re/im out)
  - per (gi, nh in 2 column halves): 2 accumulating matmuls K=108, M=48,
    N=512 into one PSUM bank; 22 tasks round-robin over 8 banks
  - drain DVE/ACT copies PSUM -> fp16 SBUF, 3 chunked DMAs out (1.08 MB)
Host post: decode, complex irfft(n=64), reshape to (B, 512, 32, 32).
"""

import sys

if "/opt/trn_rl_repo" not in sys.path:
    sys.path.insert(0, "/opt/trn_rl_repo")

import numpy as np

B, C, H, W_IMG = 8, 256, 32, 32
L = H * W_IMG               # 1024
BLK = 64
Q, P = 36, 8
NF = 33                     # rfft freqs of a 64-block
NG = 11                     # groups of 3 freqs
KROWS = 3 * Q               # 108 contraction rows per pass
MOUT = 48                   # 3 fl * (re,im) * 8 p
N_CORES = 8

_CACHE = {}


def _build_nc():
    import concourse.bacc as bacc
    import concourse.tile as tile
    import concourse.mybir as mybir

    dt = mybir.dt
    f16 = dt.float16
    f32 = dt.float32
    nc = bacc.Bacc("TRN2", target_bir_lowering=False, debug=False)

    xh = nc.dram_tensor("xh", [2 * NG * KROWS, L], f16, kind="ExternalInput").ap()
    wh = nc.dram_tensor("wh", [2 * NG * KROWS, MOUT], f16, kind="ExternalInput").ap()
    out = nc.dram_tensor("out", [MOUT, NG, L], f16, kind="ExternalOutput").ap()

    with tile.TileContext(nc) as tc:
        with (
            tc.tile_pool(name="wpool", bufs=1) as wpool,
            tc.tile_pool(name="spool", bufs=1) as spool,
            tc.tile_pool(name="opool", bufs=1) as opool,
            tc.tile_pool(name="ppool", bufs=1, space="PSUM") as ppool,
        ):
            wz = wpool.tile([128, 512], f16, name="wz", tag="wz")
            nc.gpsimd.memset(wz[:], 0.0)

            psums = [
                ppool.tile([128, 512], f32, name=f"ps{i}", tag=f"ps{i}")
                for i in range(8)
            ]

            xall = spool.tile([KROWS, 2 * NG, L], f16, name="xall", tag="xall")
            wall = wpool.tile([KROWS, 2 * NG, MOUT], f16, name="wall", tag="wall")
            osb = opool.tile([MOUT, NG, L], f16, name="osb", tag="osb")

            # PE warmup (HAM clock ramp) while the first DMAs are in flight
            for _ in range(8):
                nc.tensor.matmul(
                    psums[7][:], wz[:, :128], wz[:], start=True, stop=True
                )

            # weights first on scalar ring; x chunks split across both rings
            nc.scalar.dma_start(
                wall[:], wh[:, :].rearrange("(g p) m -> p g m", p=KROWS)
            )
            xsrc = xh[:, :].rearrange("(g p) n -> p g n", p=KROWS)
            nc.sync.dma_start(xall[:, 0:2, :], xsrc[:, 0:2, :])
            nc.sync.dma_start(xall[:, 2:8, :], xsrc[:, 2:8, :])
            nc.scalar.dma_start(xall[:, 8:15, :], xsrc[:, 8:15, :])
            nc.sync.dma_start(xall[:, 15:22, :], xsrc[:, 15:22, :])

            # out DMA chunks by gi range, issued as their drains complete
            ochunks = {3: (0, 4), 7: (4, 8), 10: (8, 11)}

            ti = 0
            for gi in range(NG):
                for nh in range(2):
                    ps = psums[ti % 8]
                    for comp in range(2):
                        g = gi * 2 + comp
                        nc.tensor.matmul(
                            ps[0:MOUT, :],
                            wall[:, g, :],
                            xall[:, g, nh * 512:(nh + 1) * 512],
                            start=(comp == 0),
                            stop=(comp == 1),
                        )
                    dst = osb[:, gi, nh * 512:(nh + 1) * 512]
                    if ti % 2 == 0:
                        nc.vector.tensor_copy(dst, ps[0:MOUT, :])
                    else:
                        nc.scalar.copy(dst, ps[0:MOUT, :])
                    ti += 1
                if gi in ochunks:
                    g0, g1 = ochunks[gi]
                    nc.sync.dma_start(out[:, g0:g1, :], osb[:, g0:g1, :])

    nc.compile()
    return nc


def _host_prep(x, weight):
    x = np.ascontiguousarray(x, dtype=np.float32)
    weight = np.ascontiguousarray(weight, dtype=np.float32)

    # shifted images -> 64-blocks (a = dd*16+m = j*36+q) -> rfft
    xp = np.pad(x, ((0, 0), (0, 0), (1, 1), (1, 1)))
    Z = np.empty((B, C, 9, 32, 32), np.float32)
    for di in range(3):
        for dj in range(3):
            Z[:, :, di * 3 + dj] = xp[:, :, di:di + 32, dj:dj + 32]
    F = np.fft.rfft(Z.reshape(B, C, 144, BLK), axis=-1)     # (B, C, 144, 33)
    Fj = F.reshape(B, C, 4, Q, NF)
    X2 = Fj.transpose(0, 4, 3, 1, 2).reshape(B, NF, Q, L)   # [b, f, q, 4c+j]
    Xhost = np.empty((B, 2 * NG, KROWS, L), np.float16)
    Xhost[:, 0::2] = X2.real.reshape(B, NG, KROWS, L)
    Xhost[:, 1::2] = X2.imag.reshape(B, NG, KROWS, L)
    Xhost = Xhost.reshape(B, 2 * NG * KROWS, L)

    wf = np.fft.rfft(weight, axis=-1)                       # (p, q, 33)
    Whost = np.zeros((NG, 2, KROWS, MOUT), np.float32)
    for gi in range(NG):
        for fl in range(3):
            f = gi * 3 + fl
            wre = wf[:, :, f].real.T                        # (q, p)
            wim = wf[:, :, f].imag.T
            ks = slice(fl * Q, fl * Q + Q)
            m0 = fl * 16
            Whost[gi, 0, ks, m0 + 0:m0 + 8] = wre
            Whost[gi, 0, ks, m0 + 8:m0 + 16] = wim
            Whost[gi, 1, ks, m0 + 0:m0 + 8] = -wim
            Whost[gi, 1, ks, m0 + 8:m0 + 16] = wre
    Whost = Whost.astype(np.float16).reshape(2 * NG * KROWS, MOUT)
    return Xhost, Whost


def _host_post(res):
    out = np.empty((B, 512, 32, 32), np.float32)
    for b in range(B):
        Hd = np.ascontiguousarray(res.results[b]["out"]).astype(np.float32)
        Hd = Hd.reshape(3, 2, 8, NG, L)
        Hc = (Hd[:, 0] + 1j * Hd[:, 1]).transpose(3, 1, 2, 0)  # (n, p, gi, fl)
        h = np.fft.irfft(Hc.reshape(L, 8, NF), n=BLK, axis=-1)  # (n, p, t)
        out[b] = h.transpose(1, 2, 0).reshape(512, 32, 32).astype(np.float32)
    return out


def _run(x, weight, trace=False, trace_kwargs=None):
    from concourse.bass_utils import run_bass_kernel_spmd

    if "nc" not in _CACHE:
        _CACHE["nc"] = _build_nc()
    nc = _CACHE["nc"]

    Xhost, Whost = _host_prep(x, weight)
    in_maps = [{"xh": Xhost[b], "wh": Whost} for b in range(N_CORES)]
    res = run_bass_kernel_spmd(
        nc,
        in_maps,
        list(range(N_CORES)),
        trace=trace,
        **(trace_kwargs or {}),
    )
    return _host_post(res), res


def kernel(x, weight):
    out, _ = _run(x, weight, trace=False)
    return out


# revision 8
# speedup vs baseline: 1.3124x; 1.0332x over previous
"""BlockCirculantConv on 8 Trainium2 NeuronCores — FFT-domain device kernel.

The reference is, per output row n = 4c+j (torch-faithful row-major reshape):
    Hf[n, p, f] = sum_q Xf[n, q, f] * wf[p, q, f]      (complex, f = 0..32)
with Xf[n, q, :] = rfft of 64-block a = 36j+q of channel c's 9 shifted images
and wf = rfft(weight). rfft/irfft are cheap O(input) transforms done on the
host; the device does only the FLOP-bearing q->p contraction (64x fewer MACs
than the expanded dense 2304x512 matmul the previous kernel used).

Device per core (data-parallel over batch, 1 image/core):
  - rhs  xall[k=(fl*36+q), g=(gi*2+comp), n=4c+j]  fp16, 4.86 MB
  - w    wall[k, g, m=(fl*16+oc*8+p)]              fp16, 0.23 MB
    (11 freq-groups gi of 3 freqs fl; comp = re/im input pass; oc = re/im out)
  - per (gi, nh in 2 column halves): 2 accumulating matmuls K=108, M=48,
    N=512 into one PSUM bank; 22 tasks round-robin over 8 banks
  - drain DVE/ACT copies PSUM -> fp16 SBUF, 3 chunked DMAs out (1.08 MB)
Host post: decode, complex irfft(n=64), reshape to (B, 512, 32, 32).
"""

import sys

if "/opt/trn_rl_repo" not in sys.path:
    sys.path.insert(0, "/opt/trn_rl_repo")

import numpy as np

B, C, H, W_IMG = 8, 256, 32, 32
L = H * W_IMG               # 1024
BLK = 64
Q, P = 36, 8
NF = 33                     # rfft freqs of a 64-block
NG = 11                     # groups of 3 freqs
KROWS = 3 * Q               # 108 contraction rows per pass
MOUT = 48                   # 3 fl * (re,im) * 8 p
N_CORES = 8

_CACHE = {}


def _build_nc():
    import concourse.bacc as bacc
    import concourse.tile as tile
    import concourse.mybir as mybir

    dt = mybir.dt
    f16 = dt.float16
    f32 = dt.float32
    nc = bacc.Bacc("TRN2", target_bir_lowering=False, debug=False)

    xh = nc.dram_tensor("xh", [KROWS, 2 * NG, L], f16, kind="ExternalInput").ap()
    wh = nc.dram_tensor("wh", [KROWS, 2 * NG, MOUT], f16, kind="ExternalInput").ap()
    out = nc.dram_tensor("out", [MOUT, NG, L], f16, kind="ExternalOutput").ap()

    with tile.TileContext(nc) as tc:
        with (
            tc.tile_pool(name="wpool", bufs=1) as wpool,
            tc.tile_pool(name="spool", bufs=1) as spool,
            tc.tile_pool(name="opool", bufs=1) as opool,
            tc.tile_pool(name="ppool", bufs=1, space="PSUM") as ppool,
        ):
            wz = wpool.tile([128, 512], f16, name="wz", tag="wz")
            nc.gpsimd.memset(wz[:], 0.0)

            psums = [
                ppool.tile([128, 512], f32, name=f"ps{i}", tag=f"ps{i}")
                for i in range(8)
            ]

            xall = spool.tile([KROWS, 2 * NG, L], f16, name="xall", tag="xall")
            wall = wpool.tile([KROWS, 2 * NG, MOUT], f16, name="wall", tag="wall")
            osb = opool.tile([MOUT, NG, L], f16, name="osb", tag="osb")

            # PE warmup (HAM clock ramp) while the first DMAs are in flight
            for _ in range(8):
                nc.tensor.matmul(
                    psums[7][:], wz[:, :128], wz[:], start=True, stop=True
                )

            # k-major host layouts: every DMA is 108 partition-lines of
            # contiguous bytes (one descriptor per line)
            nc.scalar.dma_start(wall[:], wh[:, :, :])
            nc.sync.dma_start(xall[:, 0:2, :], xh[:, 0:2, :])
            nc.sync.dma_start(xall[:, 2:8, :], xh[:, 2:8, :])
            nc.scalar.dma_start(xall[:, 8:15, :], xh[:, 8:15, :])
            nc.sync.dma_start(xall[:, 15:22, :], xh[:, 15:22, :])

            # out DMA chunks by gi range, issued as their drains complete
            ochunks = {3: (0, 4), 7: (4, 8), 10: (8, 11)}

            ti = 0
            for gi in range(NG):
                for nh in range(2):
                    ps = psums[ti % 8]
                    for comp in range(2):
                        g = gi * 2 + comp
                        nc.tensor.matmul(
                            ps[0:MOUT, :],
                            wall[:, g, :],
                            xall[:, g, nh * 512:(nh + 1) * 512],
                            start=(comp == 0),
                            stop=(comp == 1),
                        )
                    dst = osb[:, gi, nh * 512:(nh + 1) * 512]
                    if ti % 2 == 0:
                        nc.vector.tensor_copy(dst, ps[0:MOUT, :])
                    else:
                        nc.scalar.copy(dst, ps[0:MOUT, :])
                    ti += 1
                if gi in ochunks:
                    g0, g1 = ochunks[gi]
                    nc.sync.dma_start(out[:, g0:g1, :], osb[:, g0:g1, :])

    nc.compile()
    return nc


def _host_prep(x, weight):
    x = np.ascontiguousarray(x, dtype=np.float32)
    weight = np.ascontiguousarray(weight, dtype=np.float32)

    # shifted images -> 64-blocks (a = dd*16+m = j*36+q) -> rfft
    xp = np.pad(x, ((0, 0), (0, 0), (1, 1), (1, 1)))
    Z = np.empty((B, C, 9, 32, 32), np.float32)
    for di in range(3):
        for dj in range(3):
            Z[:, :, di * 3 + dj] = xp[:, :, di:di + 32, dj:dj + 32]
    F = np.fft.rfft(Z.reshape(B, C, 144, BLK), axis=-1)     # (B, C, 144, 33)
    Fj = F.reshape(B, C, 4, Q, NF)
    X2 = Fj.transpose(0, 4, 3, 1, 2).reshape(B, NF, Q, L)   # [b, f, q, 4c+j]
    X3 = X2.reshape(B, NG, 3, Q, L)                         # [b, gi, fl, q, n]
    # k-major: Xhost[b, k=(fl*36+q), g=(gi*2+comp), n]
    X4 = np.stack((X3.real, X3.imag), axis=4)               # [b,gi,fl,q,comp,n]
    Xhost = np.ascontiguousarray(
        X4.transpose(0, 2, 3, 1, 4, 5), dtype=np.float16
    ).reshape(B, KROWS, 2 * NG, L)

    wf = np.fft.rfft(weight, axis=-1)                       # (p, q, 33)
    Whost = np.zeros((NG, 2, KROWS, MOUT), np.float32)
    for gi in range(NG):
        for fl in range(3):
            f = gi * 3 + fl
            wre = wf[:, :, f].real.T                        # (q, p)
            wim = wf[:, :, f].imag.T
            ks = slice(fl * Q, fl * Q + Q)
            m0 = fl * 16
            Whost[gi, 0, ks, m0 + 0:m0 + 8] = wre
            Whost[gi, 0, ks, m0 + 8:m0 + 16] = wim
            Whost[gi, 1, ks, m0 + 0:m0 + 8] = -wim
            Whost[gi, 1, ks, m0 + 8:m0 + 16] = wre
    # k-major: Whost[k, g=(gi*2+comp), m]
    Whost = np.ascontiguousarray(
        Whost.transpose(2, 0, 1, 3), dtype=np.float16
    ).reshape(KROWS, 2 * NG, MOUT)
    return Xhost, Whost


def _host_post(res):
    out = np.empty((B, 512, 32, 32), np.float32)
    for b in range(B):
        Hd = np.ascontiguousarray(res.results[b]["out"]).astype(np.float32)
        Hd = Hd.reshape(3, 2, 8, NG, L)
        Hc = (Hd[:, 0] + 1j * Hd[:, 1]).transpose(3, 1, 2, 0)  # (n, p, gi, fl)
        h = np.fft.irfft(Hc.reshape(L, 8, NF), n=BLK, axis=-1)  # (n, p, t)
        out[b] = h.transpose(1, 2, 0).reshape(512, 32, 32).astype(np.float32)
    return out


def _run(x, weight, trace=False, trace_kwargs=None):
    from concourse.bass_utils import run_bass_kernel_spmd

    if "nc" not in _CACHE:
        _CACHE["nc"] = _build_nc()
    nc = _CACHE["nc"]

    Xhost, Whost = _host_prep(x, weight)
    in_maps = [{"xh": Xhost[b], "wh": Whost} for b in range(N_CORES)]
    res = run_bass_kernel_spmd(
        nc,
        in_maps,
        list(range(N_CORES)),
        trace=trace,
        **(trace_kwargs or {}),
    )
    return _host_post(res), res


def kernel(x, weight):
    out, _ = _run(x, weight, trace=False)
    return out
